# revision 14
# baseline (speedup 1.0000x reference)
"""Trainium2 Bass kernel for batched EEG masking-preserve-order (ragged gather).

Contract: kernel(x, noise, lengths) takes FULL inputs (N=64, L=512, D=840),
shards the batch over 8 NeuronCores (8 samples each), runs a Bass/Tile
kernel per core, and reassembles full-shape outputs:
  (masked_x [N,L,D], masked_attention_mask [N,L],
   masked_attention_mask_invert [N,L], removed_mask [N,L])

Per-sample algorithm (validated bit-exact vs the jax reference):
  clipl  = max(lengths, 16);  len_keep = floor(clipl/2)
  cand_i = i < clipl-1
  veff_i = cand_i ? noise_i : 9.0
  rank_i = #{j : veff_j < veff_i}        (no ties among candidates)
  keep_i = cand_i & (rank_i < len_keep)
  c_i    = inclusive prefix sum of keep  (PE matmuls w/ triangular ones)
  src_j  = #{i : c_i <= j}               (position of (j+1)-th kept row;
                                          = 512 OOB when j >= len_keep)
  out[j] = x[src_j] for j < len_keep, else 0.
Heavy data movement: per (sample, 128-row output chunk) one indirect-DMA
gather with [128,1] row offsets (OOB rows skipped via bounds_check; tile
pre-zeroed on the Scalar engine) + one plain contiguous store. Output rows
256..511 are never written and rely on pre-zeroed ExternalOutput buffers.
"""

import numpy as np

N, L, D = 64, 512, 840
NCORES = 8
NS = N // NCORES        # samples per core
P = 128
NCH = L // P            # position chunks per sample
JCH = 2                 # output-row chunks per sample (len_keep <= 256)
G = 2                   # samples per prefix-matmul batch
NGRP = NS // G

_nc_cache = {}


def build_nc():
    import concourse.bass as bass
    import concourse.bacc as bacc
    import concourse.mybir as mybir
    from concourse.tile import TileContext

    f32 = mybir.dt.float32
    i32 = mybir.dt.int32
    Alu = mybir.AluOpType
    Act = mybir.ActivationFunctionType

    nc = bacc.Bacc()

    x_d = nc.dram_tensor("x", [NS * L, D], f32, kind="ExternalInput")
    noise_d = nc.dram_tensor("noise", [NS, L], f32, kind="ExternalInput")
    len_d = nc.dram_tensor("lengths", [NS], i32, kind="ExternalInput")
    ident_d = nc.dram_tensor("ident", [P, P], f32, kind="ExternalInput")
    triu_d = nc.dram_tensor("triu", [P, P], f32, kind="ExternalInput")
    iota4_d = nc.dram_tensor("iota4", [P, NCH], f32, kind="ExternalInput")
    jrow_d = nc.dram_tensor("jrow", [P, JCH * P], f32, kind="ExternalInput")
    iotar_d = nc.dram_tensor("iotar", [NS, L], f32, kind="ExternalInput")

    ox_d = nc.dram_tensor("out_x", [NS * L, D], f32, kind="ExternalOutput")
    om_d = nc.dram_tensor("out_m", [NS, L], f32, kind="ExternalOutput")
    omi_d = nc.dram_tensor("out_mi", [NS, L], f32, kind="ExternalOutput")
    orm_d = nc.dram_tensor("out_r", [NS, L], f32, kind="ExternalOutput")

    with TileContext(nc) as tc:
        with (
            tc.tile_pool(name="const", bufs=1) as cp,
            tc.tile_pool(name="work", bufs=3) as wp,
            tc.tile_pool(name="gp", bufs=3) as gp,
            tc.tile_pool(name="psb", bufs=2, space="PSUM") as psb,
            tc.tile_pool(name="pss", bufs=2, space="PSUM") as pss,
        ):
            # ---------- setup: consts + per-sample scalars ----------
            ident_t = cp.tile([P, P], f32, tag="ident")
            nc.sync.dma_start(out=ident_t[:, :], in_=ident_d[:, :])
            triu_t = cp.tile([P, P], f32, tag="triu")
            nc.sync.dma_start(out=triu_t[:, :], in_=triu_d[:, :])
            iota4_t = cp.tile([P, NCH], f32, tag="iota4")
            nc.sync.dma_start(out=iota4_t[:, :], in_=iota4_d[:, :])
            jrow_t = cp.tile([P, JCH * P], f32, tag="jrow")
            nc.sync.dma_start(out=jrow_t[:, :], in_=jrow_d[:, :])
            iotar_t = cp.tile([NS, L], f32, tag="iotar")
            nc.sync.dma_start(out=iotar_t[:, :], in_=iotar_d[:, :])
            noise_t = cp.tile([NS, L], f32, tag="noise")
            nc.sync.dma_start(out=noise_t[:, :], in_=noise_d[:, :])
            lenc_i = cp.tile([NS, 1], i32, tag="lenc_i")
            nc.sync.dma_start(out=lenc_i[:, :], in_=len_d[:, None])
            lenr_i = cp.tile([1, NS], i32, tag="lenr_i")
            nc.sync.dma_start(out=lenr_i[:, :], in_=len_d[None, :])

            ones1 = cp.tile([1, P], f32, tag="ones1")
            nc.vector.memset(ones1[:, :], 1.0)
            ones128 = cp.tile([P, P], f32, tag="ones128")
            nc.vector.memset(ones128[:, :], 1.0)
            zero840 = cp.tile([P, D], f32, tag="zero840")
            nc.vector.memset(zero840[:, :], 0.0)

            # column (per-sample-on-partition) scalars
            lenc_f = cp.tile([NS, 1], f32, tag="lenc_f")
            nc.vector.tensor_copy(out=lenc_f[:, :], in_=lenc_i[:, :])
            clipl_c = cp.tile([NS, 1], f32, tag="clipl_c")
            nc.vector.tensor_scalar(
                out=clipl_c[:, :], in0=lenc_f[:, :],
                scalar1=16.0, scalar2=None, op0=Alu.max)
            thr_c = cp.tile([NS, 1], f32, tag="thr_c")
            nc.vector.tensor_scalar(
                out=thr_c[:, :], in0=clipl_c[:, :],
                scalar1=0.5, scalar2=-0.75, op0=Alu.mult, op1=Alu.add)
            clm1_c = cp.tile([NS, 1], f32, tag="clm1_c")
            nc.vector.tensor_scalar(
                out=clm1_c[:, :], in0=clipl_c[:, :],
                scalar1=-1.0, scalar2=None, op0=Alu.add)

            # row (partition-0) scalars for PE broadcast
            lenr_f = cp.tile([1, NS], f32, tag="lenr_f")
            nc.vector.tensor_copy(out=lenr_f[:, :], in_=lenr_i[:, :])
            clipl_r = cp.tile([1, NS], f32, tag="clipl_r")
            nc.vector.tensor_scalar(
                out=clipl_r[:, :], in0=lenr_f[:, :],
                scalar1=16.0, scalar2=None, op0=Alu.max)
            thr_r = cp.tile([1, NS], f32, tag="thr_r")
            nc.vector.tensor_scalar(
                out=thr_r[:, :], in0=clipl_r[:, :],
                scalar1=0.5, scalar2=-0.75, op0=Alu.mult, op1=Alu.add)
            clm1_r = cp.tile([1, NS], f32, tag="clm1_r")
            nc.vector.tensor_scalar(
                out=clm1_r[:, :], in0=clipl_r[:, :],
                scalar1=-1.0, scalar2=None, op0=Alu.add)

            # veff rows [NS, L]
            cand_r = cp.tile([NS, L], f32, tag="cand_r")
            nc.vector.tensor_tensor(
                out=cand_r[:, :], in0=iotar_t[:, :],
                in1=clm1_c[:, :].to_broadcast([NS, L]), op=Alu.is_lt)
            veff = cp.tile([NS, L], f32, tag="veff")
            nc.vector.tensor_scalar(
                out=veff[:, :], in0=noise_t[:, :],
                scalar1=-9.0, scalar2=None, op0=Alu.add)
            nc.vector.tensor_tensor(
                out=veff[:, :], in0=veff[:, :], in1=cand_r[:, :], op=Alu.mult)
            nc.vector.tensor_scalar(
                out=veff[:, :], in0=veff[:, :],
                scalar1=9.0, scalar2=None, op0=Alu.add)

            # attention mask rows + invert (independent of noise)
            attn_rows = cp.tile([NS, L], f32, tag="attn_rows")
            nc.vector.tensor_tensor(
                out=attn_rows[:, :], in0=iotar_t[:, :],
                in1=thr_c[:, :].to_broadcast([NS, L]), op=Alu.is_lt)
            nc.sync.dma_start(out=om_d[:, :], in_=attn_rows[:, :])
            inv_rows = cp.tile([NS, L], f32, tag="inv_rows")
            nc.vector.tensor_scalar(
                out=inv_rows[:, :], in0=attn_rows[:, :],
                scalar1=-1.0, scalar2=1.0, op0=Alu.mult, op1=Alu.add)
            nc.sync.dma_start(out=omi_d[:, :], in_=inv_rows[:, :])

            # broadcast per-sample scalars to [P, NS] via K=1 outer products
            bc_ps = pss.tile([P, 3 * NS], f32, tag="smallmm")
            nc.tensor.matmul(out=bc_ps[:, 0:NS], lhsT=ones1[:, :],
                             rhs=thr_r[:, :], start=True, stop=True)
            nc.tensor.matmul(out=bc_ps[:, NS:2 * NS], lhsT=ones1[:, :],
                             rhs=clipl_r[:, :], start=True, stop=True)
            nc.tensor.matmul(out=bc_ps[:, 2 * NS:3 * NS], lhsT=ones1[:, :],
                             rhs=clm1_r[:, :], start=True, stop=True)
            bc_sb = cp.tile([P, 3 * NS], f32, tag="bc_sb")
            nc.vector.tensor_copy(out=bc_sb[:, :], in_=bc_ps[:, :])
            thr_b = bc_sb[:, 0:NS]
            clipl_b = bc_sb[:, NS:2 * NS]
            clm1_b = bc_sb[:, 2 * NS:3 * NS]

            # veff rows staged on partition 0 (PE rhs needs base partition 0)
            veff_st = cp.tile([1, NS * L], f32, tag="veff_st")
            for n in range(NS):
                nc.sync.dma_start(
                    out=veff_st[0:1, n * L:(n + 1) * L],
                    in_=veff[n:n + 1, :])

            # veff as columns: transpose each [NS,128] slice -> [128,NS]
            vcolT = cp.tile([P, NCH * NS], f32, tag="vcolT")
            for c in range(NCH):
                vtr = pss.tile([P, NS], f32, tag="smallmm")
                nc.tensor.transpose(
                    out=vtr[:, :], in_=veff[:, c * P:(c + 1) * P],
                    identity=ident_t[:NS, :NS])
                nc.vector.tensor_copy(
                    out=vcolT[:, c * NS:(c + 1) * NS], in_=vtr[:, :])

            # ---------- per-sample / per-group main pipeline ----------
            for g in range(NGRP):
                keep_g = wp.tile([P, NCH * G], f32, tag="keep_g")
                for s in range(G):
                    n = g * G + s
                    # replicate veff row n across partitions
                    v_ps = psb.tile([P, L], f32, tag="vps")
                    nc.tensor.matmul(
                        out=v_ps[:, :], lhsT=ones1[:, :],
                        rhs=veff_st[0:1, n * L:(n + 1) * L],
                        start=True, stop=True)
                    # rank by counting: rank[p] = #{j: veff[j] < veff_col[p]}
                    rank_n = wp.tile([P, NCH], f32, tag="rank_n")
                    for c in range(NCH):
                        scr = wp.tile([P, L], f32, tag="scr")
                        nc.vector.tensor_scalar(
                            out=scr[:, :], in0=v_ps[:, :],
                            scalar1=vcolT[:, c * NS + n:c * NS + n + 1],
                            scalar2=None, op0=Alu.is_lt, op1=Alu.add,
                            accum_out=rank_n[:, c:c + 1])
                    # keep = (rank < len_keep) & (pos < clipl-1)
                    kk = wp.tile([P, NCH], f32, tag="kk")
                    nc.vector.tensor_tensor(
                        out=kk[:, :], in0=rank_n[:, :],
                        in1=thr_b[:, n:n + 1].to_broadcast([P, NCH]),
                        op=Alu.is_lt)
                    cand4 = wp.tile([P, NCH], f32, tag="cand4")
                    nc.vector.tensor_tensor(
                        out=cand4[:, :], in0=iota4_t[:, :],
                        in1=clm1_b[:, n:n + 1].to_broadcast([P, NCH]),
                        op=Alu.is_lt)
                    # chunk-major layout: col k*G+s
                    nc.vector.tensor_tensor(
                        out=keep_g[:, s::G], in0=kk[:, :], in1=cand4[:, :],
                        op=Alu.mult)
                    # removed mask column-form: (pos < clipl) - keep
                    inclc = wp.tile([P, NCH], f32, tag="inclc")
                    nc.vector.tensor_tensor(
                        out=inclc[:, :], in0=iota4_t[:, :],
                        in1=clipl_b[:, n:n + 1].to_broadcast([P, NCH]),
                        op=Alu.is_lt)
                    remc = wp.tile([P, NCH], f32, tag="remc")
                    nc.vector.tensor_tensor(
                        out=remc[:, :], in0=inclc[:, :], in1=keep_g[:, s::G],
                        op=Alu.subtract)
                    nc.sync.dma_start(
                        out=orm_d[n:n + 1, :].rearrange(
                            "o (c p) -> (o p) c", p=P),
                        in_=remc[:, :])

                # inclusive prefix sums over positions (both samples at once)
                c_ps = psb.tile([P, NCH * G], f32, tag="cps")
                for m in range(NCH):
                    for k in range(m + 1):
                        nc.tensor.matmul(
                            out=c_ps[:, m * G:(m + 1) * G],
                            lhsT=(triu_t[:, :] if k == m else ones128[:, :]),
                            rhs=keep_g[:, k * G:(k + 1) * G],
                            start=(k == 0), stop=(k == m))

                for s in range(G):
                    n = g * G + s
                    # invert the compaction: src_j = #{i: c_i <= j}
                    # M_ic[p, j] = (c[ic*128+p] <= j); column sums via PE
                    src_ps = pss.tile([1, JCH * P], f32, tag="srcrow")
                    for ic in range(NCH):
                        mic = wp.tile([P, JCH * P], f32, tag="mic")
                        nc.vector.tensor_tensor(
                            out=mic[:, :],
                            in0=c_ps[:, ic * G + s:ic * G + s + 1]
                                .to_broadcast([P, JCH * P]),
                            in1=jrow_t[:, :], op=Alu.is_le)
                        nc.tensor.matmul(
                            out=src_ps[:, :], lhsT=ones128[:, 0:1],
                            rhs=mic[:, :],
                            start=(ic == 0), stop=(ic == NCH - 1))
                    src_sb = wp.tile([1, JCH * P], f32, tag="src_sb")
                    nc.vector.tensor_copy(out=src_sb[:, :], in_=src_ps[:, :])

                    for jc in range(JCH):
                        # transpose [1,128] row slice -> [128,1] column
                        sc_ps = pss.tile([P, 1], f32, tag="smallmm")
                        nc.tensor.transpose(
                            out=sc_ps[:, :],
                            in_=src_sb[0:1, jc * P:(jc + 1) * P],
                            identity=ident_t[:1, :1])
                        srci = wp.tile([P, 1], i32, tag="srci")
                        nc.vector.tensor_copy(out=srci[:, :], in_=sc_ps[:, :])

                        gtile = gp.tile([P, D], f32, tag="gtile")
                        nc.scalar.activation(
                            out=gtile[:, :], in_=zero840[:, :],
                            func=Act.Copy)
                        nc.gpsimd.indirect_dma_start(
                            out=gtile[:, :],
                            out_offset=None,
                            in_=x_d[:, :],
                            in_offset=bass.IndirectOffsetOnAxis(
                                ap=srci[:, :], axis=0),
                            element_offset=n * L * D,
                            bounds_check=L - 1,
                            oob_is_err=False,
                        )
                        nc.sync.dma_start(
                            out=ox_d[n * L + jc * P:n * L + (jc + 1) * P, :],
                            in_=gtile[:, :])

    nc.finalize()
    return nc


def _consts():
    p = np.arange(P, dtype=np.float32)
    iota4 = np.stack([p + c * P for c in range(NCH)], axis=1)
    jrow = np.tile(np.arange(JCH * P, dtype=np.float32)[None, :], (P, 1))
    iotar = np.tile(np.arange(L, dtype=np.float32)[None, :], (NS, 1))
    return {
        "ident": np.eye(P, dtype=np.float32),
        "triu": np.triu(np.ones((P, P), np.float32)),
        "iota4": np.ascontiguousarray(iota4),
        "jrow": np.ascontiguousarray(jrow),
        "iotar": np.ascontiguousarray(iotar),
    }


def _get_nc():
    if "nc" not in _nc_cache:
        _nc_cache["nc"] = build_nc()
    return _nc_cache["nc"]


def make_in_maps(x, noise, lengths):
    consts = _consts()
    in_maps = []
    for ci in range(NCORES):
        sl = slice(ci * NS, (ci + 1) * NS)
        in_maps.append({
            "x": np.ascontiguousarray(
                x[sl], np.float32).reshape(NS * L, D),
            "noise": np.ascontiguousarray(noise[sl], np.float32),
            "lengths": np.ascontiguousarray(lengths[sl], np.int32),
            **consts,
        })
    return in_maps


def assemble(results):
    mx = np.concatenate(
        [r["out_x"].reshape(NS, L, D) for r in results], axis=0)
    m = np.concatenate([r["out_m"] for r in results], axis=0)
    mi = np.concatenate([r["out_mi"] for r in results], axis=0)
    rm = np.concatenate([r["out_r"] for r in results], axis=0)
    return mx, m, mi, rm


def kernel(x, noise, lengths, trace=False):
    from concourse.bass_utils import run_bass_kernel_spmd

    nc = _get_nc()
    in_maps = make_in_maps(x, noise, lengths)
    res = run_bass_kernel_spmd(nc, in_maps, list(range(NCORES)), trace=trace)
    out = assemble(res.results)
    if trace:
        return out, res
    return out
